# revision 30
# baseline (speedup 1.0000x reference)
"""Trainium2 Bass kernel for nn_MidLoss (segment-mean MSE loss).

Reference computation:
    seg_ids = repeat(arange(S), lengths)          # [N]
    means   = segment_sum(x, seg_ids) / lengths   # [S, D]
    loss    = mean((means[seg_ids] - x)**2)       # scalar

Algebraic identity used (per segment s, rows x_i):
    sum_i ||x_i - mu_s||^2 = sum_i ||x_i||^2 - ||colsum_s||^2 / L_s
so the loss needs only two sufficient statistics, computable in ONE pass:
    SSQ   = sum of x^2 over everything
    corr  = sum_s ||colsum_s / sqrt(L_s)||^2
    loss  = (SSQ - corr) / (N * D)

Distribution: rows are sharded across 8 NeuronCores at segment boundaries
(each core owns whole segments).  Each core computes a partial
(SSQ_c - corr_c) on device; the scalar all-reduce is done on host.

Per-core device pipeline (memory-bound; one pass over the data):
  - SWDGE DMA streams x fp32 HBM -> bf16 SBUF supertiles (cast in-DMA, RNE)
  - "lean" impl (default): a handful of instructions per supertile so the
    NEFF text stays tiny (~1 iram fetch).  Big NEFFs cost real time here:
    instruction fetch shares DMA engine 64 with the data stream, and every
    16 KiB fetch chunk stalls that engine ~930 ns; the old all-matmul
    version (4.6k instructions, 267 KiB text) lost ~15 us of stream time
    to it.  Per supertile [128 x G*D]:
      * ScalarE  Square activation with accum_out -> per-partition SSQ
      * VectorE  strided tensor_reduce over g     -> P[p,d] (f32 colsums
                 per partition; segments never split partitions since all
                 lengths % G == 0)
      * TensorE  one small f32 matmul memb^T @ P  -> per-segment colsums
                 (memb cols are per-segment indicators scaled 1/sqrt(L))
      * VectorE  square + reduce of that          -> corr partials
    Requires every supertile boundary to coincide with a segment boundary
    (true for the graded 384/640 alternation: 4 pairs per 4096-row
    supertile) and a uniform per-supertile segment count.
  - "gram" impl (fallback): per 128-row group X, accumulate X^T X (Gram;
    diagonal gives SSQ) and X^T M in PSUM via TensorE matmuls.
  - endgame: reduce the partials, one [1,1] dot on PE, DMA the scalar out.
"""

import os
import sys

for _p in ("/opt/trn_rl_repo", "/root/.axon_site/_ro/trn_rl_repo"):
    if os.path.isdir(_p) and _p not in sys.path:
        sys.path.insert(0, _p)

import numpy as np
import ml_dtypes

import concourse.bacc as bacc
import concourse.tile as tile
from concourse import mybir
from concourse.bass_utils import run_bass_kernel_spmd

N_CORES = 8
D = 128
IMPL = os.environ.get("MIDLOSS_IMPL", "lean")  # lean | gram
# cast mode for the gram fallback: "dma" = SWDGE casts fp32->bf16 in-DMA.
CAST_MODE = os.environ.get("MIDLOSS_CAST_MODE", "dma")
# rows per SBUF partition line (contiguous bytes per partition per supertile).
# G=32 -> 16 KiB HBM reads per descriptor; measured best for the stream.
G_CANDIDATES = (32, 16, 8, 64, 4, 128, 2, 1)
XBUFS = int(os.environ.get("MIDLOSS_XBUFS", "5"))


def _structure(lengths, n_cores=N_CORES):
    """Host-side plan: shard segments, pick layout, build membership info.

    Returns (plan, fallback) where fallback=True means shards are not
    structurally identical and SPMD with one NEFF is impossible.
    """
    lengths = np.asarray(lengths, dtype=np.int64)
    S = int(lengths.shape[0])
    offs = np.zeros(S + 1, dtype=np.int64)
    np.cumsum(lengths, out=offs[1:])
    N = int(offs[-1])

    # split at segment boundaries nearest to c*N/n_cores
    splits = [0]
    for c in range(1, n_cores):
        target = c * N / n_cores
        s = int(np.argmin(np.abs(offs - target)))
        splits.append(s)
    splits.append(S)
    for c in range(n_cores):
        if splits[c + 1] <= splits[c]:
            return None, True  # empty shard; bail to fallback
    shard_rows = [int(offs[splits[c + 1]] - offs[splits[c]]) for c in range(n_cores)]
    if len(set(shard_rows)) != 1:
        return None, True
    R = shard_rows[0]

    # largest G with all lengths % G == 0 and R % (128*G) == 0
    g_pref = int(os.environ.get("MIDLOSS_G", "0"))
    G = None
    for g in ((g_pref,) if g_pref else ()) + G_CANDIDATES:
        if R % (128 * g) == 0 and np.all(lengths % g == 0):
            G = g
            break
    if G is None:
        return None, True
    rows_super = 128 * G
    n_super = R // rows_super

    # per-core supertile structure
    cores = []
    for c in range(n_cores):
        s_lo, s_hi = splits[c], splits[c + 1]
        seg_off = offs[s_lo:s_hi + 1] - offs[s_lo]   # local boundaries [0..R]
        seg_len = lengths[s_lo:s_hi]
        s_count = s_hi - s_lo
        inv_sqrt_l = (1.0 / np.sqrt(seg_len.astype(np.float64))).astype(np.float32)

        supers = []   # (s0_local, k, memb_col_off)
        memb_cols = []  # list of [128] float32 columns
        col_off = 0
        aligned = True
        for n in range(n_super):
            lo, hi = n * rows_super, (n + 1) * rows_super
            # segments overlapping [lo, hi)
            s0 = int(np.searchsorted(seg_off, lo, side="right") - 1)
            s1 = int(np.searchsorted(seg_off, hi, side="left") - 1)
            if seg_off[s0] != lo:
                aligned = False  # segment spans a supertile boundary
            k = s1 - s0 + 1
            # partition p covers rows [lo + G*p, lo + G*(p+1))
            pstart = lo + G * np.arange(128, dtype=np.int64)
            pseg = np.searchsorted(seg_off, pstart, side="right") - 1  # [128]
            for j in range(k):
                col = np.where(pseg == s0 + j, inv_sqrt_l[s0 + j], 0.0)
                memb_cols.append(col.astype(np.float32))
            supers.append((s0, k, col_off))
            col_off += k
        memb = np.stack(memb_cols, axis=1)  # [128, C]
        cores.append(dict(s_lo=s_lo, s_hi=s_hi, s_count=s_count,
                          supers=supers, memb=memb, aligned=aligned,
                          row_lo=int(offs[s_lo]), row_hi=int(offs[s_hi])))

    # SPMD uniformity: (s0,k,col_off) lists and seg counts must match
    sig0 = (cores[0]["s_count"], tuple(cores[0]["supers"]))
    for c in range(1, n_cores):
        if (cores[c]["s_count"], tuple(cores[c]["supers"])) != sig0:
            return None, True
    s_count = cores[0]["s_count"]
    if s_count > 512:  # psum_cs must fit one bank region per matmul slice
        return None, True

    ks = [k for (_s0, k, _c0) in cores[0]["supers"]]
    lean_ok = (all(core["aligned"] for core in cores)
               and len(set(ks)) == 1 and ks[0] <= 128)

    plan = dict(R=R, G=G, n_super=n_super, s_count=s_count,
                n_memb_cols=cores[0]["memb"].shape[1],
                supers=cores[0]["supers"], cores=cores, N=N,
                lean_ok=lean_ok, k_uniform=ks[0] if len(set(ks)) == 1 else 0)
    return plan, False


def _build_nc_lean(R, G, n_super, n_memb_cols, supers, k_uniform):
    """Tiny-NEFF implementation: ~7 instructions per supertile."""
    f32 = mybir.dt.float32
    bf16 = mybir.dt.bfloat16
    Sq = mybir.ActivationFunctionType.Square
    AX = mybir.AxisListType.X
    ADD = mybir.AluOpType.add
    K = k_uniform

    nc = bacc.Bacc()
    x = nc.dram_tensor("x", [R, D], f32, kind="ExternalInput")
    memb = nc.dram_tensor("memb", [128, n_memb_cols], f32, kind="ExternalInput")
    y = nc.dram_tensor("y", [1, 1], f32, kind="ExternalOutput")

    FB = G * D
    with tile.TileContext(nc) as tc:
        with (
            tc.tile_pool(name="xbf", bufs=XBUFS) as xbf_pool,
            tc.tile_pool(name="dmy", bufs=2) as dummy_pool,
            tc.tile_pool(name="sqd", bufs=2) as sq_pool,
            tc.tile_pool(name="sqd2", bufs=2) as sq2_pool,
            tc.tile_pool(name="tree", bufs=2) as tree_pool,
            tc.tile_pool(name="pcol", bufs=3) as p_pool,
            tc.tile_pool(name="ssq2", bufs=2) as ssq2_pool,
            tc.tile_pool(name="singles", bufs=1) as singles,
            tc.tile_pool(name="psum", bufs=2, space="PSUM") as psum_pool,
        ):
            # split the per-supertile square: ScalarE (activation w/ accum)
            # takes FA cols, DVE (mul+reduce) the rest, so neither engine
            # paces the DMA stream; ScalarE alone would (~4.6us serial vs a
            # ~5.2us/supertile stream at G=32).
            alpha = float(os.environ.get(
                "MIDLOSS_ALPHA", "0.8125" if FB >= 8192 else "0.75"))
            FA = min(FB, max(D, (int(FB * alpha) // 512) * 512)) \
                if FB >= 4 * D else FB
            H = FB // 2
            # the last supertile's consumers are pure tail (they run after
            # the final DMA packet); loading it in column halves lets the
            # first half's square/tree work overlap the stream
            tail_split = FB >= 8 * D and FA > H

            memb_sb = singles.tile([128, n_memb_cols], f32, tag="memb")
            nc.sync.dma_start(out=memb_sb[:], in_=memb[:])
            # ssq cols [0,n_super) filled by ScalarE, [n_super,2*n_super) by
            # DVE, plus one spill col for the split last supertile
            n_ssq = n_super + (n_super if FA < FB else 0) + (1 if tail_split else 0)
            ssq_acc = singles.tile([128, n_ssq], f32, tag="ssq")
            cs_sq = singles.tile([K, n_super], f32, tag="cssq")

            xv = x[:].rearrange("(n p g) d -> n p (g d)", p=128, g=G)
            for n in range(n_super):
                last = tail_split and n == n_super - 1
                xb = xbf_pool.tile([128, FB], bf16)
                # 1-descriptor dummy load: SWDGE descriptors round-robin over
                # the 16 DMA engines CONTINUOUSLY across instructions, and a
                # supertile is 128 = 8x16 lines, so without this each engine
                # is pinned to a fixed HBM-address-residue class all run.
                # All 8 SPMD cores read congruent addresses in phase, so an
                # engine owning a hot residue runs ~25% slow for the whole
                # stream and drags the kernel (observed: one engine 198us
                # busy vs 158us for the other 15).  The extra descriptor
                # drifts the binding by one engine per supertile, spreading
                # the hot class over all engines.  Costs one 16 KiB re-read
                # (+0.8% traffic).
                dummy = dummy_pool.tile([1, FB], bf16)
                nc.gpsimd.dma_start(out=dummy[:], in_=xv[n, 0:1])
                if last:
                    nc.gpsimd.dma_start(out=xb[:, 0:H], in_=xv[n, :, 0:H])
                    nc.gpsimd.dma_start(out=xb[:, H:FB], in_=xv[n, :, H:FB])
                else:
                    nc.gpsimd.dma_start(out=xb[:], in_=xv[n])
                if n == 0:
                    # Per-core ring decorrelation: all cores run the same
                    # NEFF, so the per-supertile drift above still rotates
                    # them in lockstep and they keep colliding on the same
                    # hot residue simultaneously.  Core c executes exactly c
                    # of these predicated 1-line loads, offsetting each
                    # core's descriptor ring differently.
                    pid = nc.gpsimd.partition_id()
                    for j in range(N_CORES - 1):
                        pdummy = dummy_pool.tile([1, FB], bf16, tag="pdmy",
                                                 bufs=1, name=f"pdummy{j}")
                        nc.gpsimd.dma_start(out=pdummy[:], in_=xv[0, 0:1],
                                            cond=pid > j, cond_hint=False)
                # per-partition sum of squares of this supertile
                sq = sq_pool.tile([128, FA], bf16)
                if last:
                    nc.scalar.activation(out=sq[:, 0:H], in_=xb[:, 0:H],
                                         func=Sq,
                                         accum_out=ssq_acc[:, n:n + 1])
                    nc.scalar.activation(out=sq[:, H:FA], in_=xb[:, H:FA],
                                         func=Sq,
                                         accum_out=ssq_acc[:, n_ssq - 1:n_ssq])
                else:
                    nc.scalar.activation(out=sq[:], in_=xb[:, 0:FA], func=Sq,
                                         accum_out=ssq_acc[:, n:n + 1])
                # per-partition column sums over the G rows (one segment
                # per partition since all lengths % G == 0).  Binary add-tree
                # over contiguous halves: a strided tensor_reduce over g runs
                # at ~1.7 cy/elem on DVE, the contiguous tree at ~0.5.
                P = p_pool.tile([128, D], f32)
                if G == 1:
                    nc.vector.tensor_copy(out=P[:], in_=xb[:])
                elif G == 2:
                    nc.vector.tensor_add(P[:], xb[:, 0:D], xb[:, D:2 * D])
                elif last:
                    # two half-trees so half A reduces while half B streams
                    h = tree_pool.tile([128, FB // 2], bf16)
                    with nc.allow_low_precision("bf16 colsum tree; corr"
                                                " term is tiny vs SSQ"):
                        for base in (0, H):
                            w = H // 2
                            nc.vector.tensor_add(h[:, 0:w], xb[:, base:base + w],
                                                 xb[:, base + w:base + 2 * w])
                            while w > 2 * D:
                                w //= 2
                                nc.vector.tensor_add(h[:, 0:w], h[:, 0:w],
                                                     h[:, w:2 * w])
                            if base == 0:
                                nc.vector.tensor_add(P[:], h[:, 0:D],
                                                     h[:, D:2 * D])
                        nc.vector.tensor_add(P[:], P[:], h[:, 0:D])
                        nc.vector.tensor_add(P[:], P[:], h[:, D:2 * D])
                else:
                    h = tree_pool.tile([128, FB // 2], bf16)
                    w = FB // 2
                    with nc.allow_low_precision("bf16 colsum tree; corr"
                                                " term is tiny vs SSQ"):
                        nc.vector.tensor_add(h[:, 0:w], xb[:, 0:w],
                                             xb[:, w:2 * w])
                        while w > 2 * D:
                            w //= 2
                            nc.vector.tensor_add(h[:, 0:w], h[:, 0:w],
                                                 h[:, w:2 * w])
                    # final level outputs f32 directly
                    nc.vector.tensor_add(P[:], h[:, 0:D], h[:, D:2 * D])
                # DVE's share of the sum-of-squares, after the tree so the
                # matmul (and ScalarE's colsum square behind it) start early
                if FA < FB:
                    sq2 = sq2_pool.tile([128, FB - FA], bf16)
                    with nc.allow_low_precision("bf16 squares; summed f32"):
                        nc.vector.tensor_mul(sq2[:], xb[:, FA:FB],
                                             xb[:, FA:FB])
                    nc.vector.tensor_reduce(
                        out=ssq_acc[:, n_super + n:n_super + n + 1],
                        in_=sq2[:], axis=AX, op=ADD)
                # fold partitions into per-segment colsums / sqrt(L)
                s0, k, c0 = supers[n]
                psum_s = psum_pool.tile([K, D], f32)
                nc.tensor.matmul(psum_s[0:k, :], lhsT=memb_sb[:, c0:c0 + k],
                                 rhs=P[:], start=True, stop=True)
                s_sq = ssq2_pool.tile([K, D], f32)
                nc.scalar.activation(out=s_sq[0:k, :], in_=psum_s[0:k, :],
                                     func=Sq, accum_out=cs_sq[0:k, n:n + 1])

            # ---- endgame (tiny) ----
            rsum = singles.tile([128, 1], f32, tag="rsum")
            nc.vector.tensor_reduce(out=rsum[:], in_=ssq_acc[:], axis=AX, op=ADD)
            rcs = singles.tile([K, 1], f32, tag="rcs")
            nc.vector.tensor_reduce(out=rcs[:], in_=cs_sq[:], axis=AX, op=ADD)
            vec = singles.tile([128, 1], f32, tag="vec")
            nc.vector.memset(vec[:], 0.0)
            nc.vector.tensor_copy(out=vec[0:K, :], in_=rcs[:])
            diff = singles.tile([128, 1], f32, tag="diff")
            nc.vector.tensor_sub(diff[:], rsum[:], vec[:])
            ones = singles.tile([128, 1], f32, tag="ones")
            nc.vector.memset(ones[:], 1.0)
            ptot = psum_pool.tile([1, 1], f32, tag="ptot")
            nc.tensor.matmul(ptot[:], lhsT=ones[:], rhs=diff[:],
                             start=True, stop=True)
            out_sb = singles.tile([1, 1], f32, tag="out")
            nc.vector.tensor_copy(out=out_sb[:], in_=ptot[:])
            nc.sync.dma_start(out=y[:], in_=out_sb[:])

    nc.compile()
    return nc


def _build_nc_gram(R, G, n_super, s_count, n_memb_cols, supers, cast_mode):
    """Fallback: Gram + membership matmuls per 128-row group."""
    f32 = mybir.dt.float32
    bf16 = mybir.dt.bfloat16

    nc = bacc.Bacc()
    x = nc.dram_tensor("x", [R, D], f32, kind="ExternalInput")
    memb = nc.dram_tensor("memb", [128, n_memb_cols], bf16, kind="ExternalInput")
    ident = nc.dram_tensor("ident", [128, 128], f32, kind="ExternalInput")
    y = nc.dram_tensor("y", [1, 1], f32, kind="ExternalOutput")

    FB = G * D  # free size of one supertile
    with tile.TileContext(nc) as tc:
        with (
            tc.tile_pool(name="xin", bufs=3) as xin_pool,
            tc.tile_pool(name="xbf", bufs=5) as xbf_pool,
            tc.tile_pool(name="singles", bufs=1) as singles,
            tc.tile_pool(name="small", bufs=1) as small,
            tc.tile_pool(name="psum", bufs=1, space="PSUM") as psum_pool,
        ):
            memb_sb = singles.tile([128, n_memb_cols], bf16)
            nc.sync.dma_start(out=memb_sb[:], in_=memb[:])
            ident_sb = singles.tile([128, 128], f32)
            nc.sync.dma_start(out=ident_sb[:], in_=ident[:])

            psum_cs = psum_pool.tile([128, s_count], f32)
            psum_gram = psum_pool.tile([128, 128], f32)

            xv = x[:].rearrange("(n p g) d -> n p (g d)", p=128, g=G)
            for n in range(n_super):
                if cast_mode == "dma":
                    xb = xbf_pool.tile([128, FB], bf16)
                    nc.gpsimd.dma_start(out=xb[:], in_=xv[n])
                else:
                    x32 = xin_pool.tile([128, FB], f32)
                    nc.sync.dma_start(out=x32[:], in_=xv[n])
                    xb = xbf_pool.tile([128, FB], bf16)
                    nc.vector.tensor_copy(out=xb[:], in_=x32[:])

                s0, k, c0 = supers[n]
                first = n == 0
                last = n == n_super - 1
                for g in range(G):
                    st = xb[:, g * D:(g + 1) * D]
                    nc.tensor.matmul(
                        psum_gram[:], lhsT=st, rhs=st,
                        start=(first and g == 0), stop=(last and g == G - 1),
                    )
                    nc.tensor.matmul(
                        psum_cs[:, s0:s0 + k], lhsT=st,
                        rhs=memb_sb[:, c0:c0 + k],
                        start=(first and g == 0), stop=(last and g == G - 1),
                    )

            # ---- endgame (tiny) ----
            # NOTE: tensor_tensor_reduce / scalar_tensor_tensor crash the HW
            # (NRT_EXEC_UNIT_UNRECOVERABLE) in this runtime even though
            # CoreSim accepts them — use plain mul + reduce instead.
            cs_sb = small.tile([128, s_count], f32)
            nc.vector.tensor_copy(out=cs_sb[:], in_=psum_cs[:])
            cs_sq = small.tile([128, s_count], f32)
            nc.vector.tensor_mul(cs_sq[:], cs_sb[:], cs_sb[:])
            r1 = small.tile([128, 1], f32)
            nc.vector.tensor_reduce(out=r1[:], in_=cs_sq[:],
                                    axis=mybir.AxisListType.X,
                                    op=mybir.AluOpType.add)
            g_mask = small.tile([128, 128], f32)
            nc.vector.tensor_mul(g_mask[:], psum_gram[:], ident_sb[:])
            r2 = small.tile([128, 1], f32)
            nc.vector.tensor_reduce(out=r2[:], in_=g_mask[:],
                                    axis=mybir.AxisListType.X,
                                    op=mybir.AluOpType.add)
            diff = small.tile([128, 1], f32)
            nc.vector.tensor_sub(diff[:], r2[:], r1[:])
            ones = small.tile([128, 1], f32)
            nc.vector.memset(ones[:], 1.0)
            ptot = psum_pool.tile([1, 1], f32)
            nc.tensor.matmul(ptot[:], lhsT=ones[:], rhs=diff[:],
                             start=True, stop=True)
            out_sb = small.tile([1, 1], f32)
            nc.vector.tensor_copy(out=out_sb[:], in_=ptot[:])
            nc.sync.dma_start(out=y[:], in_=out_sb[:])

    nc.compile()
    return nc


_CACHE = {}


def _impl_for(plan):
    return IMPL if (IMPL != "lean" or plan["lean_ok"]) else "gram"


def _get_nc(plan):
    impl = _impl_for(plan)
    key = (impl, plan["R"], plan["G"], plan["n_super"], plan["s_count"],
           plan["n_memb_cols"], tuple(plan["supers"]))
    nc = _CACHE.get(key)
    if nc is None:
        if impl == "lean":
            nc = _build_nc_lean(plan["R"], plan["G"], plan["n_super"],
                                plan["n_memb_cols"], plan["supers"],
                                plan["k_uniform"])
        else:
            nc = _build_nc_gram(plan["R"], plan["G"], plan["n_super"],
                                plan["s_count"], plan["n_memb_cols"],
                                plan["supers"], CAST_MODE)
        _CACHE[key] = nc
    return nc


def _run_spmd(plan, x_np, trace=False):
    impl = _impl_for(plan)
    nc = _get_nc(plan)
    ident = np.eye(128, dtype=np.float32)
    in_maps = []
    for c in range(N_CORES):
        info = plan["cores"][c]
        shard = np.ascontiguousarray(x_np[info["row_lo"]:info["row_hi"]])
        if impl == "lean":
            in_maps.append({"x": shard, "memb": info["memb"]})
        else:
            in_maps.append({
                "x": shard,
                "memb": info["memb"].astype(ml_dtypes.bfloat16),
                "ident": ident,
            })
    last_err = None
    for attempt in range(3):
        try:
            res = run_bass_kernel_spmd(nc, in_maps,
                                       core_ids=list(range(N_CORES)),
                                       trace=trace)
            break
        except Exception as e:  # rare transient device-unrecoverable flakes
            last_err = e
    else:
        raise last_err
    partials = [float(res.results[c]["y"][0, 0]) for c in range(N_CORES)]
    return partials, res


def _numpy_fallback(x_np, lengths):
    """Pure-host fallback for input structures the SPMD path can't express.

    (Never expected for the graded problem sizes; kept for robustness.)"""
    lengths = np.asarray(lengths, dtype=np.int64)
    offs = np.concatenate([[0], np.cumsum(lengths)])
    x = x_np.astype(np.float64)
    ssq = float((x * x).sum())
    corr = 0.0
    for s in range(len(lengths)):
        cs = x[offs[s]:offs[s + 1]].sum(axis=0)
        corr += float((cs * cs).sum()) / float(lengths[s])
    return np.float32((ssq - corr) / x.size)


def kernel(inputs, lengths):
    x_np = np.asarray(inputs, dtype=np.float32)
    lengths_np = np.asarray(lengths)
    plan, fallback = _structure(lengths_np)
    if fallback:
        return _numpy_fallback(x_np, lengths_np)
    partials, _ = _run_spmd(plan, x_np)
    total = float(np.sum(np.asarray(partials, dtype=np.float64)))
    loss = total / (plan["N"] * D)
    return np.asarray(loss, dtype=np.float32)


# revision 31
# speedup vs baseline: 1.2280x; 1.2280x over previous
"""Trainium2 Bass kernel for nn_MidLoss (segment-mean MSE loss).

Reference computation:
    seg_ids = repeat(arange(S), lengths)          # [N]
    means   = segment_sum(x, seg_ids) / lengths   # [S, D]
    loss    = mean((means[seg_ids] - x)**2)       # scalar

Algebraic identity used (per segment s, rows x_i):
    sum_i ||x_i - mu_s||^2 = sum_i ||x_i||^2 - ||colsum_s||^2 / L_s
so the loss needs only two sufficient statistics, computable in ONE pass:
    SSQ   = sum of x^2 over everything
    corr  = sum_s ||colsum_s / sqrt(L_s)||^2
    loss  = (SSQ - corr) / (N * D)

Distribution: rows are sharded across 8 NeuronCores at segment boundaries
(each core owns whole segments).  Each core computes a partial
(SSQ_c - corr_c) on device; the scalar all-reduce is done on host.

Per-core device pipeline (memory-bound; one pass over the data):
  - SWDGE DMA streams x fp32 HBM -> bf16 SBUF supertiles (cast in-DMA, RNE)
  - "lean" impl (default): a handful of instructions per supertile so the
    NEFF text stays tiny (~1 iram fetch).  Big NEFFs cost real time here:
    instruction fetch shares DMA engine 64 with the data stream, and every
    16 KiB fetch chunk stalls that engine ~930 ns; the old all-matmul
    version (4.6k instructions, 267 KiB text) lost ~15 us of stream time
    to it.  Per supertile [128 x G*D]:
      * ScalarE  Square activation with accum_out -> per-partition SSQ
      * VectorE  strided tensor_reduce over g     -> P[p,d] (f32 colsums
                 per partition; segments never split partitions since all
                 lengths % G == 0)
      * TensorE  one small f32 matmul memb^T @ P  -> per-segment colsums
                 (memb cols are per-segment indicators scaled 1/sqrt(L))
      * VectorE  square + reduce of that          -> corr partials
    Requires every supertile boundary to coincide with a segment boundary
    (true for the graded 384/640 alternation: 4 pairs per 4096-row
    supertile) and a uniform per-supertile segment count.
  - "gram" impl (fallback): per 128-row group X, accumulate X^T X (Gram;
    diagonal gives SSQ) and X^T M in PSUM via TensorE matmuls.
  - endgame: reduce the partials, one [1,1] dot on PE, DMA the scalar out.
"""

import os
import sys

for _p in ("/opt/trn_rl_repo", "/root/.axon_site/_ro/trn_rl_repo"):
    if os.path.isdir(_p) and _p not in sys.path:
        sys.path.insert(0, _p)

import numpy as np
import ml_dtypes

import concourse.bacc as bacc
import concourse.tile as tile
from concourse import mybir
from concourse.bass_utils import run_bass_kernel_spmd

N_CORES = 8
D = 128
IMPL = os.environ.get("MIDLOSS_IMPL", "lean")  # lean | gram
# cast mode for the gram fallback: "dma" = SWDGE casts fp32->bf16 in-DMA.
CAST_MODE = os.environ.get("MIDLOSS_CAST_MODE", "dma")
# rows per SBUF partition line (contiguous bytes per partition per supertile).
# G=32 -> 16 KiB HBM reads per descriptor; measured best for the stream.
G_CANDIDATES = (32, 16, 8, 64, 4, 128, 2, 1)
XBUFS = int(os.environ.get("MIDLOSS_XBUFS", "5"))


def _structure(lengths, n_cores=N_CORES):
    """Host-side plan: shard segments, pick layout, build membership info.

    Returns (plan, fallback) where fallback=True means shards are not
    structurally identical and SPMD with one NEFF is impossible.
    """
    lengths = np.asarray(lengths, dtype=np.int64)
    S = int(lengths.shape[0])
    offs = np.zeros(S + 1, dtype=np.int64)
    np.cumsum(lengths, out=offs[1:])
    N = int(offs[-1])

    # split at segment boundaries nearest to c*N/n_cores
    splits = [0]
    for c in range(1, n_cores):
        target = c * N / n_cores
        s = int(np.argmin(np.abs(offs - target)))
        splits.append(s)
    splits.append(S)
    for c in range(n_cores):
        if splits[c + 1] <= splits[c]:
            return None, True  # empty shard; bail to fallback
    shard_rows = [int(offs[splits[c + 1]] - offs[splits[c]]) for c in range(n_cores)]
    if len(set(shard_rows)) != 1:
        return None, True
    R = shard_rows[0]

    # largest G with all lengths % G == 0 and R % (128*G) == 0
    g_pref = int(os.environ.get("MIDLOSS_G", "0"))
    G = None
    for g in ((g_pref,) if g_pref else ()) + G_CANDIDATES:
        if R % (128 * g) == 0 and np.all(lengths % g == 0):
            G = g
            break
    if G is None:
        return None, True
    rows_super = 128 * G
    n_super = R // rows_super

    # per-core supertile structure
    cores = []
    for c in range(n_cores):
        s_lo, s_hi = splits[c], splits[c + 1]
        seg_off = offs[s_lo:s_hi + 1] - offs[s_lo]   # local boundaries [0..R]
        seg_len = lengths[s_lo:s_hi]
        s_count = s_hi - s_lo
        inv_sqrt_l = (1.0 / np.sqrt(seg_len.astype(np.float64))).astype(np.float32)

        supers = []   # (s0_local, k, memb_col_off)
        memb_cols = []  # list of [128] float32 columns
        col_off = 0
        aligned = True
        for n in range(n_super):
            lo, hi = n * rows_super, (n + 1) * rows_super
            # segments overlapping [lo, hi)
            s0 = int(np.searchsorted(seg_off, lo, side="right") - 1)
            s1 = int(np.searchsorted(seg_off, hi, side="left") - 1)
            if seg_off[s0] != lo:
                aligned = False  # segment spans a supertile boundary
            k = s1 - s0 + 1
            # partition p covers rows [lo + G*p, lo + G*(p+1))
            pstart = lo + G * np.arange(128, dtype=np.int64)
            pseg = np.searchsorted(seg_off, pstart, side="right") - 1  # [128]
            for j in range(k):
                col = np.where(pseg == s0 + j, inv_sqrt_l[s0 + j], 0.0)
                memb_cols.append(col.astype(np.float32))
            supers.append((s0, k, col_off))
            col_off += k
        memb = np.stack(memb_cols, axis=1)  # [128, C]
        cores.append(dict(s_lo=s_lo, s_hi=s_hi, s_count=s_count,
                          supers=supers, memb=memb, aligned=aligned,
                          row_lo=int(offs[s_lo]), row_hi=int(offs[s_hi])))

    # SPMD uniformity: (s0,k,col_off) lists and seg counts must match
    sig0 = (cores[0]["s_count"], tuple(cores[0]["supers"]))
    for c in range(1, n_cores):
        if (cores[c]["s_count"], tuple(cores[c]["supers"])) != sig0:
            return None, True
    s_count = cores[0]["s_count"]
    if s_count > 512:  # psum_cs must fit one bank region per matmul slice
        return None, True

    ks = [k for (_s0, k, _c0) in cores[0]["supers"]]
    lean_ok = (all(core["aligned"] for core in cores)
               and len(set(ks)) == 1 and ks[0] <= 128)

    plan = dict(R=R, G=G, n_super=n_super, s_count=s_count,
                n_memb_cols=cores[0]["memb"].shape[1],
                supers=cores[0]["supers"], cores=cores, N=N,
                lean_ok=lean_ok, k_uniform=ks[0] if len(set(ks)) == 1 else 0)
    return plan, False


def _build_nc_lean(R, G, n_super, n_memb_cols, supers, k_uniform):
    """Tiny-NEFF implementation: ~7 instructions per supertile."""
    f32 = mybir.dt.float32
    bf16 = mybir.dt.bfloat16
    Sq = mybir.ActivationFunctionType.Square
    AX = mybir.AxisListType.X
    ADD = mybir.AluOpType.add
    K = k_uniform

    nc = bacc.Bacc()
    x = nc.dram_tensor("x", [R, D], f32, kind="ExternalInput")
    memb = nc.dram_tensor("memb", [128, n_memb_cols], f32, kind="ExternalInput")
    y = nc.dram_tensor("y", [1, 1], f32, kind="ExternalOutput")

    FB = G * D
    with tile.TileContext(nc) as tc:
        with (
            tc.tile_pool(name="xbf", bufs=XBUFS) as xbf_pool,
            tc.tile_pool(name="dmy", bufs=2) as dummy_pool,
            tc.tile_pool(name="sqd", bufs=2) as sq_pool,
            tc.tile_pool(name="sqd2", bufs=2) as sq2_pool,
            tc.tile_pool(name="tree", bufs=2) as tree_pool,
            tc.tile_pool(name="pcol", bufs=3) as p_pool,
            tc.tile_pool(name="ssq2", bufs=2) as ssq2_pool,
            tc.tile_pool(name="singles", bufs=1) as singles,
            tc.tile_pool(name="psum", bufs=2, space="PSUM") as psum_pool,
        ):
            memb_sb = singles.tile([128, n_memb_cols], f32, tag="memb")
            nc.sync.dma_start(out=memb_sb[:], in_=memb[:])
            # ssq cols [0,n_super) filled by ScalarE, [n_super,2*n_super) by DVE
            ssq_acc = singles.tile([128, 2 * n_super], f32, tag="ssq")
            cs_sq = singles.tile([K, n_super], f32, tag="cssq")

            # split the per-supertile square: ScalarE (activation w/ accum)
            # takes FA cols, DVE (mul+reduce) the rest, so neither engine
            # paces the DMA stream; ScalarE alone would (~4.6us serial vs a
            # ~5.2us/supertile stream at G=32).
            alpha = float(os.environ.get(
                "MIDLOSS_ALPHA", "0.8125" if FB >= 8192 else "0.75"))
            FA = min(FB, max(D, (int(FB * alpha) // 512) * 512)) \
                if FB >= 4 * D else FB

            xv = x[:].rearrange("(n p g) d -> n p (g d)", p=128, g=G)
            for n in range(n_super):
                xb = xbf_pool.tile([128, FB], bf16)
                # 1-descriptor dummy load: SWDGE descriptors round-robin over
                # the 16 DMA engines CONTINUOUSLY across instructions, and a
                # supertile is 128 = 8x16 lines, so without this each engine
                # is pinned to a fixed HBM-address-residue class all run.
                # All 8 SPMD cores read congruent addresses in phase, so an
                # engine owning a hot residue runs ~25% slow for the whole
                # stream and drags the kernel (observed: one engine 198us
                # busy vs 158us for the other 15).  The extra descriptor
                # drifts the binding by one engine per supertile, spreading
                # the hot class over all engines.  Costs one 16 KiB re-read
                # (+0.8% traffic).
                dummy = dummy_pool.tile([1, FB], bf16)
                nc.gpsimd.dma_start(out=dummy[:], in_=xv[n, 0:1])
                nc.gpsimd.dma_start(out=xb[:], in_=xv[n])
                if n == 0:
                    # Per-core ring decorrelation: all cores run the same
                    # NEFF, so the per-supertile drift above still rotates
                    # them in lockstep and they keep colliding on the same
                    # hot residue simultaneously.  Core c executes exactly c
                    # of these predicated 1-line loads, offsetting each
                    # core's descriptor ring differently.
                    pid = nc.gpsimd.partition_id()
                    for j in range(N_CORES - 1):
                        pdummy = dummy_pool.tile([1, FB], bf16, tag="pdmy",
                                                 bufs=1, name=f"pdummy{j}")
                        nc.gpsimd.dma_start(out=pdummy[:], in_=xv[0, 0:1],
                                            cond=pid > j, cond_hint=False)
                # per-partition sum of squares of this supertile
                sq = sq_pool.tile([128, FA], bf16)
                nc.scalar.activation(out=sq[:], in_=xb[:, 0:FA], func=Sq,
                                     accum_out=ssq_acc[:, n:n + 1])
                # per-partition column sums over the G rows (one segment
                # per partition since all lengths % G == 0).  Binary add-tree
                # over contiguous halves: a strided tensor_reduce over g runs
                # at ~1.7 cy/elem on DVE, the contiguous tree at ~0.5.
                P = p_pool.tile([128, D], f32)
                if G == 1:
                    nc.vector.tensor_copy(out=P[:], in_=xb[:])
                elif G == 2:
                    nc.vector.tensor_add(P[:], xb[:, 0:D], xb[:, D:2 * D])
                else:
                    h = tree_pool.tile([128, FB // 2], bf16)
                    w = FB // 2
                    with nc.allow_low_precision("bf16 colsum tree; corr"
                                                " term is tiny vs SSQ"):
                        nc.vector.tensor_add(h[:, 0:w], xb[:, 0:w],
                                             xb[:, w:2 * w])
                        while w > 2 * D:
                            w //= 2
                            nc.vector.tensor_add(h[:, 0:w], h[:, 0:w],
                                                 h[:, w:2 * w])
                    # final level outputs f32 directly
                    nc.vector.tensor_add(P[:], h[:, 0:D], h[:, D:2 * D])
                # DVE's share of the sum-of-squares, after the tree so the
                # matmul (and ScalarE's colsum square behind it) start early
                if FA < FB:
                    sq2 = sq2_pool.tile([128, FB - FA], bf16)
                    with nc.allow_low_precision("bf16 squares; summed f32"):
                        nc.vector.tensor_mul(sq2[:], xb[:, FA:FB],
                                             xb[:, FA:FB])
                    nc.vector.tensor_reduce(
                        out=ssq_acc[:, n_super + n:n_super + n + 1],
                        in_=sq2[:], axis=AX, op=ADD)
                # fold partitions into per-segment colsums / sqrt(L)
                s0, k, c0 = supers[n]
                psum_s = psum_pool.tile([K, D], f32)
                nc.tensor.matmul(psum_s[0:k, :], lhsT=memb_sb[:, c0:c0 + k],
                                 rhs=P[:], start=True, stop=True)
                s_sq = ssq2_pool.tile([K, D], f32)
                nc.scalar.activation(out=s_sq[0:k, :], in_=psum_s[0:k, :],
                                     func=Sq, accum_out=cs_sq[0:k, n:n + 1])

            # ---- endgame (tiny) ----
            rsum = singles.tile([128, 1], f32, tag="rsum")
            nc.vector.tensor_reduce(out=rsum[:], in_=ssq_acc[:], axis=AX, op=ADD)
            rcs = singles.tile([K, 1], f32, tag="rcs")
            nc.vector.tensor_reduce(out=rcs[:], in_=cs_sq[:], axis=AX, op=ADD)
            vec = singles.tile([128, 1], f32, tag="vec")
            nc.vector.memset(vec[:], 0.0)
            nc.vector.tensor_copy(out=vec[0:K, :], in_=rcs[:])
            diff = singles.tile([128, 1], f32, tag="diff")
            nc.vector.tensor_sub(diff[:], rsum[:], vec[:])
            ones = singles.tile([128, 1], f32, tag="ones")
            nc.vector.memset(ones[:], 1.0)
            ptot = psum_pool.tile([1, 1], f32, tag="ptot")
            nc.tensor.matmul(ptot[:], lhsT=ones[:], rhs=diff[:],
                             start=True, stop=True)
            out_sb = singles.tile([1, 1], f32, tag="out")
            nc.vector.tensor_copy(out=out_sb[:], in_=ptot[:])
            nc.sync.dma_start(out=y[:], in_=out_sb[:])

    nc.compile()
    return nc


def _build_nc_gram(R, G, n_super, s_count, n_memb_cols, supers, cast_mode):
    """Fallback: Gram + membership matmuls per 128-row group."""
    f32 = mybir.dt.float32
    bf16 = mybir.dt.bfloat16

    nc = bacc.Bacc()
    x = nc.dram_tensor("x", [R, D], f32, kind="ExternalInput")
    memb = nc.dram_tensor("memb", [128, n_memb_cols], bf16, kind="ExternalInput")
    ident = nc.dram_tensor("ident", [128, 128], f32, kind="ExternalInput")
    y = nc.dram_tensor("y", [1, 1], f32, kind="ExternalOutput")

    FB = G * D  # free size of one supertile
    with tile.TileContext(nc) as tc:
        with (
            tc.tile_pool(name="xin", bufs=3) as xin_pool,
            tc.tile_pool(name="xbf", bufs=5) as xbf_pool,
            tc.tile_pool(name="singles", bufs=1) as singles,
            tc.tile_pool(name="small", bufs=1) as small,
            tc.tile_pool(name="psum", bufs=1, space="PSUM") as psum_pool,
        ):
            memb_sb = singles.tile([128, n_memb_cols], bf16)
            nc.sync.dma_start(out=memb_sb[:], in_=memb[:])
            ident_sb = singles.tile([128, 128], f32)
            nc.sync.dma_start(out=ident_sb[:], in_=ident[:])

            psum_cs = psum_pool.tile([128, s_count], f32)
            psum_gram = psum_pool.tile([128, 128], f32)

            xv = x[:].rearrange("(n p g) d -> n p (g d)", p=128, g=G)
            for n in range(n_super):
                if cast_mode == "dma":
                    xb = xbf_pool.tile([128, FB], bf16)
                    nc.gpsimd.dma_start(out=xb[:], in_=xv[n])
                else:
                    x32 = xin_pool.tile([128, FB], f32)
                    nc.sync.dma_start(out=x32[:], in_=xv[n])
                    xb = xbf_pool.tile([128, FB], bf16)
                    nc.vector.tensor_copy(out=xb[:], in_=x32[:])

                s0, k, c0 = supers[n]
                first = n == 0
                last = n == n_super - 1
                for g in range(G):
                    st = xb[:, g * D:(g + 1) * D]
                    nc.tensor.matmul(
                        psum_gram[:], lhsT=st, rhs=st,
                        start=(first and g == 0), stop=(last and g == G - 1),
                    )
                    nc.tensor.matmul(
                        psum_cs[:, s0:s0 + k], lhsT=st,
                        rhs=memb_sb[:, c0:c0 + k],
                        start=(first and g == 0), stop=(last and g == G - 1),
                    )

            # ---- endgame (tiny) ----
            # NOTE: tensor_tensor_reduce / scalar_tensor_tensor crash the HW
            # (NRT_EXEC_UNIT_UNRECOVERABLE) in this runtime even though
            # CoreSim accepts them — use plain mul + reduce instead.
            cs_sb = small.tile([128, s_count], f32)
            nc.vector.tensor_copy(out=cs_sb[:], in_=psum_cs[:])
            cs_sq = small.tile([128, s_count], f32)
            nc.vector.tensor_mul(cs_sq[:], cs_sb[:], cs_sb[:])
            r1 = small.tile([128, 1], f32)
            nc.vector.tensor_reduce(out=r1[:], in_=cs_sq[:],
                                    axis=mybir.AxisListType.X,
                                    op=mybir.AluOpType.add)
            g_mask = small.tile([128, 128], f32)
            nc.vector.tensor_mul(g_mask[:], psum_gram[:], ident_sb[:])
            r2 = small.tile([128, 1], f32)
            nc.vector.tensor_reduce(out=r2[:], in_=g_mask[:],
                                    axis=mybir.AxisListType.X,
                                    op=mybir.AluOpType.add)
            diff = small.tile([128, 1], f32)
            nc.vector.tensor_sub(diff[:], r2[:], r1[:])
            ones = small.tile([128, 1], f32)
            nc.vector.memset(ones[:], 1.0)
            ptot = psum_pool.tile([1, 1], f32)
            nc.tensor.matmul(ptot[:], lhsT=ones[:], rhs=diff[:],
                             start=True, stop=True)
            out_sb = small.tile([1, 1], f32)
            nc.vector.tensor_copy(out=out_sb[:], in_=ptot[:])
            nc.sync.dma_start(out=y[:], in_=out_sb[:])

    nc.compile()
    return nc


_CACHE = {}


def _impl_for(plan):
    return IMPL if (IMPL != "lean" or plan["lean_ok"]) else "gram"


def _get_nc(plan):
    impl = _impl_for(plan)
    key = (impl, plan["R"], plan["G"], plan["n_super"], plan["s_count"],
           plan["n_memb_cols"], tuple(plan["supers"]))
    nc = _CACHE.get(key)
    if nc is None:
        if impl == "lean":
            nc = _build_nc_lean(plan["R"], plan["G"], plan["n_super"],
                                plan["n_memb_cols"], plan["supers"],
                                plan["k_uniform"])
        else:
            nc = _build_nc_gram(plan["R"], plan["G"], plan["n_super"],
                                plan["s_count"], plan["n_memb_cols"],
                                plan["supers"], CAST_MODE)
        _CACHE[key] = nc
    return nc


def _run_spmd(plan, x_np, trace=False):
    impl = _impl_for(plan)
    nc = _get_nc(plan)
    ident = np.eye(128, dtype=np.float32)
    in_maps = []
    for c in range(N_CORES):
        info = plan["cores"][c]
        shard = np.ascontiguousarray(x_np[info["row_lo"]:info["row_hi"]])
        if impl == "lean":
            in_maps.append({"x": shard, "memb": info["memb"]})
        else:
            in_maps.append({
                "x": shard,
                "memb": info["memb"].astype(ml_dtypes.bfloat16),
                "ident": ident,
            })
    last_err = None
    for attempt in range(3):
        try:
            res = run_bass_kernel_spmd(nc, in_maps,
                                       core_ids=list(range(N_CORES)),
                                       trace=trace)
            break
        except Exception as e:  # rare transient device-unrecoverable flakes
            last_err = e
    else:
        raise last_err
    partials = [float(res.results[c]["y"][0, 0]) for c in range(N_CORES)]
    return partials, res


def _numpy_fallback(x_np, lengths):
    """Pure-host fallback for input structures the SPMD path can't express.

    (Never expected for the graded problem sizes; kept for robustness.)"""
    lengths = np.asarray(lengths, dtype=np.int64)
    offs = np.concatenate([[0], np.cumsum(lengths)])
    x = x_np.astype(np.float64)
    ssq = float((x * x).sum())
    corr = 0.0
    for s in range(len(lengths)):
        cs = x[offs[s]:offs[s + 1]].sum(axis=0)
        corr += float((cs * cs).sum()) / float(lengths[s])
    return np.float32((ssq - corr) / x.size)


def kernel(inputs, lengths):
    x_np = np.asarray(inputs, dtype=np.float32)
    lengths_np = np.asarray(lengths)
    plan, fallback = _structure(lengths_np)
    if fallback:
        return _numpy_fallback(x_np, lengths_np)
    partials, _ = _run_spmd(plan, x_np)
    total = float(np.sum(np.asarray(partials, dtype=np.float64)))
    loss = total / (plan["N"] * D)
    return np.asarray(loss, dtype=np.float32)


# revision 34
# speedup vs baseline: 1.2414x; 1.0109x over previous
"""Trainium2 Bass kernel for nn_MidLoss (segment-mean MSE loss).

Reference computation:
    seg_ids = repeat(arange(S), lengths)          # [N]
    means   = segment_sum(x, seg_ids) / lengths   # [S, D]
    loss    = mean((means[seg_ids] - x)**2)       # scalar

Algebraic identity used (per segment s, rows x_i):
    sum_i ||x_i - mu_s||^2 = sum_i ||x_i||^2 - ||colsum_s||^2 / L_s
so the loss needs only two sufficient statistics, computable in ONE pass:
    SSQ   = sum of x^2 over everything
    corr  = sum_s ||colsum_s / sqrt(L_s)||^2
    loss  = (SSQ - corr) / (N * D)

Distribution: rows are sharded across 8 NeuronCores at segment boundaries
(each core owns whole segments).  Each core computes a partial
(SSQ_c - corr_c) on device; the scalar all-reduce is done on host.

Per-core device pipeline (memory-bound; one pass over the data):
  - SWDGE DMA streams x fp32 HBM -> bf16 SBUF supertiles (cast in-DMA, RNE)
  - "lean" impl (default): a handful of instructions per supertile so the
    NEFF text stays tiny (~1 iram fetch).  Big NEFFs cost real time here:
    instruction fetch shares DMA engine 64 with the data stream, and every
    16 KiB fetch chunk stalls that engine ~930 ns; the old all-matmul
    version (4.6k instructions, 267 KiB text) lost ~15 us of stream time
    to it.  Per supertile [128 x G*D]:
      * ScalarE  Square activation with accum_out -> per-partition SSQ
      * VectorE  strided tensor_reduce over g     -> P[p,d] (f32 colsums
                 per partition; segments never split partitions since all
                 lengths % G == 0)
      * TensorE  one small f32 matmul memb^T @ P  -> per-segment colsums
                 (memb cols are per-segment indicators scaled 1/sqrt(L))
      * VectorE  square + reduce of that          -> corr partials
    Requires every supertile boundary to coincide with a segment boundary
    (true for the graded 384/640 alternation: 4 pairs per 4096-row
    supertile) and a uniform per-supertile segment count.
  - "gram" impl (fallback): per 128-row group X, accumulate X^T X (Gram;
    diagonal gives SSQ) and X^T M in PSUM via TensorE matmuls.
  - endgame: reduce the partials, one [1,1] dot on PE, DMA the scalar out.
"""

import os
import sys

for _p in ("/opt/trn_rl_repo", "/root/.axon_site/_ro/trn_rl_repo"):
    if os.path.isdir(_p) and _p not in sys.path:
        sys.path.insert(0, _p)

import numpy as np
import ml_dtypes

import concourse.bacc as bacc
import concourse.tile as tile
from concourse import mybir
from concourse.bass_utils import run_bass_kernel_spmd

N_CORES = 8
D = 128
IMPL = os.environ.get("MIDLOSS_IMPL", "lean")  # lean | gram
# cast mode for the gram fallback: "dma" = SWDGE casts fp32->bf16 in-DMA.
CAST_MODE = os.environ.get("MIDLOSS_CAST_MODE", "dma")
# rows per SBUF partition line (contiguous bytes per partition per supertile).
# G=32 -> 16 KiB HBM reads per descriptor; measured best for the stream.
G_CANDIDATES = (32, 16, 8, 64, 4, 128, 2, 1)
XBUFS = int(os.environ.get("MIDLOSS_XBUFS", "5"))


def _structure(lengths, n_cores=N_CORES):
    """Host-side plan: shard segments, pick layout, build membership info.

    Returns (plan, fallback) where fallback=True means shards are not
    structurally identical and SPMD with one NEFF is impossible.
    """
    lengths = np.asarray(lengths, dtype=np.int64)
    S = int(lengths.shape[0])
    offs = np.zeros(S + 1, dtype=np.int64)
    np.cumsum(lengths, out=offs[1:])
    N = int(offs[-1])

    # split at segment boundaries nearest to c*N/n_cores
    splits = [0]
    for c in range(1, n_cores):
        target = c * N / n_cores
        s = int(np.argmin(np.abs(offs - target)))
        splits.append(s)
    splits.append(S)
    for c in range(n_cores):
        if splits[c + 1] <= splits[c]:
            return None, True  # empty shard; bail to fallback
    shard_rows = [int(offs[splits[c + 1]] - offs[splits[c]]) for c in range(n_cores)]
    if len(set(shard_rows)) != 1:
        return None, True
    R = shard_rows[0]

    # largest G with all lengths % G == 0 and R % (128*G) == 0
    g_pref = int(os.environ.get("MIDLOSS_G", "0"))
    G = None
    for g in ((g_pref,) if g_pref else ()) + G_CANDIDATES:
        if R % (128 * g) == 0 and np.all(lengths % g == 0):
            G = g
            break
    if G is None:
        return None, True
    rows_super = 128 * G
    n_super = R // rows_super

    # per-core supertile structure
    cores = []
    for c in range(n_cores):
        s_lo, s_hi = splits[c], splits[c + 1]
        seg_off = offs[s_lo:s_hi + 1] - offs[s_lo]   # local boundaries [0..R]
        seg_len = lengths[s_lo:s_hi]
        s_count = s_hi - s_lo
        inv_sqrt_l = (1.0 / np.sqrt(seg_len.astype(np.float64))).astype(np.float32)

        supers = []   # (s0_local, k, memb_col_off)
        memb_cols = []  # list of [128] float32 columns
        col_off = 0
        aligned = True
        for n in range(n_super):
            lo, hi = n * rows_super, (n + 1) * rows_super
            # segments overlapping [lo, hi)
            s0 = int(np.searchsorted(seg_off, lo, side="right") - 1)
            s1 = int(np.searchsorted(seg_off, hi, side="left") - 1)
            if seg_off[s0] != lo:
                aligned = False  # segment spans a supertile boundary
            k = s1 - s0 + 1
            # partition p covers rows [lo + G*p, lo + G*(p+1))
            pstart = lo + G * np.arange(128, dtype=np.int64)
            pseg = np.searchsorted(seg_off, pstart, side="right") - 1  # [128]
            for j in range(k):
                col = np.where(pseg == s0 + j, inv_sqrt_l[s0 + j], 0.0)
                memb_cols.append(col.astype(np.float32))
            supers.append((s0, k, col_off))
            col_off += k
        memb = np.stack(memb_cols, axis=1)  # [128, C]
        cores.append(dict(s_lo=s_lo, s_hi=s_hi, s_count=s_count,
                          supers=supers, memb=memb, aligned=aligned,
                          row_lo=int(offs[s_lo]), row_hi=int(offs[s_hi])))

    # SPMD uniformity: (s0,k,col_off) lists and seg counts must match
    sig0 = (cores[0]["s_count"], tuple(cores[0]["supers"]))
    for c in range(1, n_cores):
        if (cores[c]["s_count"], tuple(cores[c]["supers"])) != sig0:
            return None, True
    s_count = cores[0]["s_count"]
    if s_count > 512:  # psum_cs must fit one bank region per matmul slice
        return None, True

    ks = [k for (_s0, k, _c0) in cores[0]["supers"]]
    lean_ok = (all(core["aligned"] for core in cores)
               and len(set(ks)) == 1 and ks[0] <= 128)

    plan = dict(R=R, G=G, n_super=n_super, s_count=s_count,
                n_memb_cols=cores[0]["memb"].shape[1],
                supers=cores[0]["supers"], cores=cores, N=N,
                lean_ok=lean_ok, k_uniform=ks[0] if len(set(ks)) == 1 else 0)
    return plan, False


def _build_nc_lean(R, G, n_super, n_memb_cols, supers, k_uniform):
    """Tiny-NEFF implementation: ~7 instructions per supertile."""
    f32 = mybir.dt.float32
    bf16 = mybir.dt.bfloat16
    Sq = mybir.ActivationFunctionType.Square
    AX = mybir.AxisListType.X
    ADD = mybir.AluOpType.add
    K = k_uniform

    nc = bacc.Bacc()
    x = nc.dram_tensor("x", [R, D], f32, kind="ExternalInput")
    memb = nc.dram_tensor("memb", [128, n_memb_cols], f32, kind="ExternalInput")
    y = nc.dram_tensor("y", [1, 1], f32, kind="ExternalOutput")

    FB = G * D
    with tile.TileContext(nc) as tc:
        with (
            tc.tile_pool(name="xbf", bufs=XBUFS) as xbf_pool,
            tc.tile_pool(name="dmy", bufs=2) as dummy_pool,
            tc.tile_pool(name="sqd", bufs=2) as sq_pool,
            tc.tile_pool(name="sqd2", bufs=2) as sq2_pool,
            tc.tile_pool(name="tree", bufs=2) as tree_pool,
            tc.tile_pool(name="pcol", bufs=3) as p_pool,
            tc.tile_pool(name="ssq2", bufs=2) as ssq2_pool,
            tc.tile_pool(name="singles", bufs=1) as singles,
            tc.tile_pool(name="psum", bufs=2, space="PSUM") as psum_pool,
        ):
            # Engine balance per supertile vs the ~5.2us/supertile stream
            # (G=32): ScalarE takes the whole x^2 (one clean dep on xb,
            # ~3.97us); DVE takes the colsum tree plus the previous
            # supertile's psum square (~3.8us).  MIDLOSS_ALPHA < 1 moves
            # (1-alpha) of the x^2 back to DVE (mul+reduce) if needed.
            alpha = float(os.environ.get("MIDLOSS_ALPHA", "1.0"))
            FA = min(FB, max(D, (int(FB * alpha) // 512) * 512)) \
                if FB >= 4 * D else FB

            memb_sb = singles.tile([128, n_memb_cols], f32, tag="memb")
            nc.sync.dma_start(out=memb_sb[:], in_=memb[:])
            # ssq cols [0,n_super) filled by ScalarE, the rest by DVE
            n_ssq = n_super + (n_super if FA < FB else 0)
            ssq_acc = singles.tile([128, n_ssq], f32, tag="ssq")
            cs_sq = singles.tile([K, n_super], f32, tag="cssq")

            xv = x[:].rearrange("(n p g) d -> n p (g d)", p=128, g=G)
            pending = None
            for n in range(n_super):
                xb = xbf_pool.tile([128, FB], bf16)
                # 1-descriptor dummy load: SWDGE descriptors round-robin over
                # the 16 DMA engines CONTINUOUSLY across instructions, and a
                # supertile is 128 = 8x16 lines, so without this each engine
                # is pinned to a fixed HBM-address-residue class all run.
                # All 8 SPMD cores read congruent addresses in phase, so an
                # engine owning a hot residue runs ~25% slow for the whole
                # stream and drags the kernel (observed: one engine 198us
                # busy vs 158us for the other 15).  The extra descriptor
                # drifts the binding by one engine per supertile, spreading
                # the hot class over all engines.  Costs one 16 KiB re-read
                # (+0.8% traffic).
                dummy = dummy_pool.tile([1, FB], bf16)
                nc.gpsimd.dma_start(out=dummy[:], in_=xv[n, 0:1])
                nc.gpsimd.dma_start(out=xb[:], in_=xv[n])
                if n == 0:
                    # Per-core ring decorrelation: all cores run the same
                    # NEFF, so the per-supertile drift above still rotates
                    # them in lockstep and they keep colliding on the same
                    # hot residue simultaneously.  Core c executes exactly c
                    # of these predicated 1-line loads, offsetting each
                    # core's descriptor ring differently.
                    pid = nc.gpsimd.partition_id()
                    for j in range(N_CORES - 1):
                        pdummy = dummy_pool.tile([1, FB], bf16, tag="pdmy",
                                                 bufs=1, name=f"pdummy{j}")
                        nc.gpsimd.dma_start(out=pdummy[:], in_=xv[0, 0:1],
                                            cond=pid > j, cond_hint=False)
                # per-partition sum of squares of this supertile
                sq = sq_pool.tile([128, FA], bf16)
                nc.scalar.activation(out=sq[:], in_=xb[:, 0:FA], func=Sq,
                                     accum_out=ssq_acc[:, n:n + 1])
                # per-partition column sums over the G rows (one segment
                # per partition since all lengths % G == 0).  Binary add-tree
                # over contiguous halves: a strided tensor_reduce over g runs
                # at ~1.7 cy/elem on DVE, the contiguous tree at ~0.5.
                P = p_pool.tile([128, D], f32)
                if G == 1:
                    nc.vector.tensor_copy(out=P[:], in_=xb[:])
                elif G == 2:
                    nc.vector.tensor_add(P[:], xb[:, 0:D], xb[:, D:2 * D])
                else:
                    h = tree_pool.tile([128, FB // 2], bf16)
                    w = FB // 2
                    with nc.allow_low_precision("bf16 colsum tree; corr"
                                                " term is tiny vs SSQ"):
                        nc.vector.tensor_add(h[:, 0:w], xb[:, 0:w],
                                             xb[:, w:2 * w])
                        while w > 2 * D:
                            w //= 2
                            nc.vector.tensor_add(h[:, 0:w], h[:, 0:w],
                                                 h[:, w:2 * w])
                    # final level outputs f32 directly
                    nc.vector.tensor_add(P[:], h[:, 0:D], h[:, D:2 * D])
                # DVE's share of the sum-of-squares, after the tree so the
                # matmul (and ScalarE's colsum square behind it) start early
                if FA < FB:
                    sq2 = sq2_pool.tile([128, FB - FA], bf16)
                    with nc.allow_low_precision("bf16 squares; summed f32"):
                        nc.vector.tensor_mul(sq2[:], xb[:, FA:FB],
                                             xb[:, FA:FB])
                    nc.vector.tensor_reduce(
                        out=ssq_acc[:, n_super + n:n_super + n + 1],
                        in_=sq2[:], axis=AX, op=ADD)
                # square+reduce the PREVIOUS supertile's per-segment colsums
                # on DVE.  One-stage software pipelining: by now that matmul
                # finished long ago, so DVE never stalls on the PE (emitted
                # in-order, it would wait ~1us per supertile).
                if pending is not None:
                    pp, pk, pn = pending
                    c_sb = ssq2_pool.tile([K, D], f32)
                    nc.vector.tensor_copy(out=c_sb[0:pk, :], in_=pp[0:pk, :])
                    nc.vector.tensor_mul(c_sb[0:pk, :], c_sb[0:pk, :],
                                         c_sb[0:pk, :])
                    nc.vector.tensor_reduce(out=cs_sq[0:pk, pn:pn + 1],
                                            in_=c_sb[0:pk, :], axis=AX, op=ADD)
                # fold partitions into per-segment colsums / sqrt(L)
                s0, k, c0 = supers[n]
                psum_s = psum_pool.tile([K, D], f32)
                nc.tensor.matmul(psum_s[0:k, :], lhsT=memb_sb[:, c0:c0 + k],
                                 rhs=P[:], start=True, stop=True)
                pending = (psum_s, k, n)

            # flush the last pending colsum square
            pp, pk, pn = pending
            c_sb = ssq2_pool.tile([K, D], f32, name="c_sb_last")
            nc.vector.tensor_copy(out=c_sb[0:pk, :], in_=pp[0:pk, :])
            nc.vector.tensor_mul(c_sb[0:pk, :], c_sb[0:pk, :], c_sb[0:pk, :])
            nc.vector.tensor_reduce(out=cs_sq[0:pk, pn:pn + 1],
                                    in_=c_sb[0:pk, :], axis=AX, op=ADD)

            # ---- endgame (tiny) ----
            rsum = singles.tile([128, 1], f32, tag="rsum")
            nc.vector.tensor_reduce(out=rsum[:], in_=ssq_acc[:], axis=AX, op=ADD)
            rcs = singles.tile([K, 1], f32, tag="rcs")
            nc.vector.tensor_reduce(out=rcs[:], in_=cs_sq[:], axis=AX, op=ADD)
            vec = singles.tile([128, 1], f32, tag="vec")
            nc.vector.memset(vec[:], 0.0)
            nc.vector.tensor_copy(out=vec[0:K, :], in_=rcs[:])
            diff = singles.tile([128, 1], f32, tag="diff")
            nc.vector.tensor_sub(diff[:], rsum[:], vec[:])
            ones = singles.tile([128, 1], f32, tag="ones")
            nc.vector.memset(ones[:], 1.0)
            ptot = psum_pool.tile([1, 1], f32, tag="ptot")
            nc.tensor.matmul(ptot[:], lhsT=ones[:], rhs=diff[:],
                             start=True, stop=True)
            out_sb = singles.tile([1, 1], f32, tag="out")
            nc.vector.tensor_copy(out=out_sb[:], in_=ptot[:])
            nc.sync.dma_start(out=y[:], in_=out_sb[:])

    nc.compile()
    return nc


def _build_nc_gram(R, G, n_super, s_count, n_memb_cols, supers, cast_mode):
    """Fallback: Gram + membership matmuls per 128-row group."""
    f32 = mybir.dt.float32
    bf16 = mybir.dt.bfloat16

    nc = bacc.Bacc()
    x = nc.dram_tensor("x", [R, D], f32, kind="ExternalInput")
    memb = nc.dram_tensor("memb", [128, n_memb_cols], bf16, kind="ExternalInput")
    ident = nc.dram_tensor("ident", [128, 128], f32, kind="ExternalInput")
    y = nc.dram_tensor("y", [1, 1], f32, kind="ExternalOutput")

    FB = G * D  # free size of one supertile
    with tile.TileContext(nc) as tc:
        with (
            tc.tile_pool(name="xin", bufs=3) as xin_pool,
            tc.tile_pool(name="xbf", bufs=5) as xbf_pool,
            tc.tile_pool(name="singles", bufs=1) as singles,
            tc.tile_pool(name="small", bufs=1) as small,
            tc.tile_pool(name="psum", bufs=1, space="PSUM") as psum_pool,
        ):
            memb_sb = singles.tile([128, n_memb_cols], bf16)
            nc.sync.dma_start(out=memb_sb[:], in_=memb[:])
            ident_sb = singles.tile([128, 128], f32)
            nc.sync.dma_start(out=ident_sb[:], in_=ident[:])

            psum_cs = psum_pool.tile([128, s_count], f32)
            psum_gram = psum_pool.tile([128, 128], f32)

            xv = x[:].rearrange("(n p g) d -> n p (g d)", p=128, g=G)
            for n in range(n_super):
                if cast_mode == "dma":
                    xb = xbf_pool.tile([128, FB], bf16)
                    nc.gpsimd.dma_start(out=xb[:], in_=xv[n])
                else:
                    x32 = xin_pool.tile([128, FB], f32)
                    nc.sync.dma_start(out=x32[:], in_=xv[n])
                    xb = xbf_pool.tile([128, FB], bf16)
                    nc.vector.tensor_copy(out=xb[:], in_=x32[:])

                s0, k, c0 = supers[n]
                first = n == 0
                last = n == n_super - 1
                for g in range(G):
                    st = xb[:, g * D:(g + 1) * D]
                    nc.tensor.matmul(
                        psum_gram[:], lhsT=st, rhs=st,
                        start=(first and g == 0), stop=(last and g == G - 1),
                    )
                    nc.tensor.matmul(
                        psum_cs[:, s0:s0 + k], lhsT=st,
                        rhs=memb_sb[:, c0:c0 + k],
                        start=(first and g == 0), stop=(last and g == G - 1),
                    )

            # ---- endgame (tiny) ----
            # NOTE: tensor_tensor_reduce / scalar_tensor_tensor crash the HW
            # (NRT_EXEC_UNIT_UNRECOVERABLE) in this runtime even though
            # CoreSim accepts them — use plain mul + reduce instead.
            cs_sb = small.tile([128, s_count], f32)
            nc.vector.tensor_copy(out=cs_sb[:], in_=psum_cs[:])
            cs_sq = small.tile([128, s_count], f32)
            nc.vector.tensor_mul(cs_sq[:], cs_sb[:], cs_sb[:])
            r1 = small.tile([128, 1], f32)
            nc.vector.tensor_reduce(out=r1[:], in_=cs_sq[:],
                                    axis=mybir.AxisListType.X,
                                    op=mybir.AluOpType.add)
            g_mask = small.tile([128, 128], f32)
            nc.vector.tensor_mul(g_mask[:], psum_gram[:], ident_sb[:])
            r2 = small.tile([128, 1], f32)
            nc.vector.tensor_reduce(out=r2[:], in_=g_mask[:],
                                    axis=mybir.AxisListType.X,
                                    op=mybir.AluOpType.add)
            diff = small.tile([128, 1], f32)
            nc.vector.tensor_sub(diff[:], r2[:], r1[:])
            ones = small.tile([128, 1], f32)
            nc.vector.memset(ones[:], 1.0)
            ptot = psum_pool.tile([1, 1], f32)
            nc.tensor.matmul(ptot[:], lhsT=ones[:], rhs=diff[:],
                             start=True, stop=True)
            out_sb = small.tile([1, 1], f32)
            nc.vector.tensor_copy(out=out_sb[:], in_=ptot[:])
            nc.sync.dma_start(out=y[:], in_=out_sb[:])

    nc.compile()
    return nc


_CACHE = {}


def _impl_for(plan):
    return IMPL if (IMPL != "lean" or plan["lean_ok"]) else "gram"


def _get_nc(plan):
    impl = _impl_for(plan)
    key = (impl, plan["R"], plan["G"], plan["n_super"], plan["s_count"],
           plan["n_memb_cols"], tuple(plan["supers"]))
    nc = _CACHE.get(key)
    if nc is None:
        if impl == "lean":
            nc = _build_nc_lean(plan["R"], plan["G"], plan["n_super"],
                                plan["n_memb_cols"], plan["supers"],
                                plan["k_uniform"])
        else:
            nc = _build_nc_gram(plan["R"], plan["G"], plan["n_super"],
                                plan["s_count"], plan["n_memb_cols"],
                                plan["supers"], CAST_MODE)
        _CACHE[key] = nc
    return nc


def _run_spmd(plan, x_np, trace=False):
    impl = _impl_for(plan)
    nc = _get_nc(plan)
    ident = np.eye(128, dtype=np.float32)
    in_maps = []
    for c in range(N_CORES):
        info = plan["cores"][c]
        shard = np.ascontiguousarray(x_np[info["row_lo"]:info["row_hi"]])
        if impl == "lean":
            in_maps.append({"x": shard, "memb": info["memb"]})
        else:
            in_maps.append({
                "x": shard,
                "memb": info["memb"].astype(ml_dtypes.bfloat16),
                "ident": ident,
            })
    last_err = None
    for attempt in range(3):
        try:
            res = run_bass_kernel_spmd(nc, in_maps,
                                       core_ids=list(range(N_CORES)),
                                       trace=trace)
            break
        except Exception as e:  # rare transient device-unrecoverable flakes
            last_err = e
    else:
        raise last_err
    partials = [float(res.results[c]["y"][0, 0]) for c in range(N_CORES)]
    return partials, res


def _numpy_fallback(x_np, lengths):
    """Pure-host fallback for input structures the SPMD path can't express.

    (Never expected for the graded problem sizes; kept for robustness.)"""
    lengths = np.asarray(lengths, dtype=np.int64)
    offs = np.concatenate([[0], np.cumsum(lengths)])
    x = x_np.astype(np.float64)
    ssq = float((x * x).sum())
    corr = 0.0
    for s in range(len(lengths)):
        cs = x[offs[s]:offs[s + 1]].sum(axis=0)
        corr += float((cs * cs).sum()) / float(lengths[s])
    return np.float32((ssq - corr) / x.size)


def kernel(inputs, lengths):
    x_np = np.asarray(inputs, dtype=np.float32)
    lengths_np = np.asarray(lengths)
    plan, fallback = _structure(lengths_np)
    if fallback:
        return _numpy_fallback(x_np, lengths_np)
    partials, _ = _run_spmd(plan, x_np)
    total = float(np.sum(np.asarray(partials, dtype=np.float64)))
    loss = total / (plan["N"] * D)
    return np.asarray(loss, dtype=np.float32)
